# revision 38
# baseline (speedup 1.0000x reference)
"""AdaptiveTokenFilter Trainium2 kernel.

kernel(**inputs) takes the FULL inputs (token_embeddings [8,4096,1024], W1
[1024,2048], b1 [2048], W2 [2048,1], b2 [1], k_logits [64]) and returns
(filtered_embeddings, selection_mask, expected_k) matching the reference.

Strategy: data-parallel over batch — one NeuronCore per batch row.
Per core: the scorer GEMM1 ([4096,1024]@[1024,2048]) runs as a 3-term
bf16 hi/lo-split matmul (fp32-level fidelity at bf16 TensorE speed; the
hi/lo split and the [D,S] transpose are host-side data prep), relu+bias
fused into the PSUM->SBUF copy, GEMM2 ([.,2048]@[2048,1]) in fp32
interleaved into the GEMM1 tile loop, gumbel perturbation added on-chip
(host-generated noise, bit-identical to the reference's jax threefry
stream), exact top-k threshold via the gpsimd kth_largest instruction,
and the filtered output produced sparsely: outputs are pre-zeroed by the
runner, selected token ids are compacted on-chip (iota/mask/sparse_gather
with OOB sentinels), and only the <=64 selected rows of X are
gather/scattered into the output via indirect DMA.

The learnable-k branch (k_selected, expected_k) is a 64-element
computation done on host; k_selected is baked into the device program.
"""

import numpy as np

S, D, H, P = 4096, 1024, 2048, 128
B = 8
NS, ND, NH, SC = 8, 8, 16, 512  # S-chunks, D-tiles, H-tiles, chunk size
MAX_K = 64
TAU = 1.0
K_TAU = 1.0

_cache = {}


def _host_prng():
    """Reproduce the reference's jax PRNG stream bit-exactly on CPU."""
    import jax
    import jax.numpy as jnp

    cpu = jax.devices("cpu")[0]
    with jax.default_device(cpu):
        rng = jax.random.key(42)
        rng1, rng2 = jax.random.split(rng)

        def _gumbel(r, shape):
            u = jax.random.uniform(r, shape, minval=1e-08, maxval=1.0 - 1e-08)
            return -jnp.log(-jnp.log(u))

        k_gumbel = np.asarray(_gumbel(rng1, (MAX_K,)))
        g2 = np.asarray(_gumbel(rng2, (B, S)))
    return k_gumbel, g2


def _build(k_selected: int, n_rep: int = 1, ablate: frozenset = frozenset()):
    import concourse.bass as bass  # noqa: F401
    import concourse.mybir as mybir
    import concourse.tile as tile
    from concourse import bacc

    dt = mybir.dt
    f32, bf16 = dt.float32, dt.bfloat16
    Alu = mybir.AluOpType

    nc = bacc.Bacc("TRN2", target_bir_lowering=False, debug=False)
    x = nc.dram_tensor("x", [S, D], f32, kind="ExternalInput").ap()
    # pre-transposed bf16 hi/lo split of x, chunk-major: [sc, p, d_outer, s_in]
    xthi_d = nc.dram_tensor("xthi", [NS, P, ND, SC], bf16, kind="ExternalInput").ap()
    xtlo_d = nc.dram_tensor("xtlo", [NS, P, ND, SC], bf16, kind="ExternalInput").ap()
    w1hi_d = nc.dram_tensor("w1hi", [P, ND, H], bf16, kind="ExternalInput").ap()
    w1lo_d = nc.dram_tensor("w1lo", [P, ND, H], bf16, kind="ExternalInput").ap()
    b1 = nc.dram_tensor("b1", [H], f32, kind="ExternalInput").ap()
    w2 = nc.dram_tensor("w2", [H], f32, kind="ExternalInput").ap()
    gum = nc.dram_tensor("gum", [S], f32, kind="ExternalInput").ap()
    out = nc.dram_tensor("out", [S, D], f32, kind="ExternalOutput").ap()
    mask_out = nc.dram_tensor("mask", [S], f32, kind="ExternalOutput").ap()

    with tile.TileContext(nc) as tc:
        with (
            tc.tile_pool(name="const", bufs=1) as cp,
            tc.tile_pool(name="dram", bufs=1, space="DRAM") as dramp,
        ):
            b1sb = cp.tile([P, NH], f32)
            nc.sync.dma_start(b1sb[:], b1.rearrange("(o p) -> p o", p=P))
            w2sb = cp.tile([P, NH], f32)
            nc.sync.dma_start(w2sb[:], w2.rearrange("(o p) -> p o", p=P))
            sel4 = cp.tile([P, 1], f32)
            nc.gpsimd.memset(sel4[:], 0.0)
            for c in range(4):
                nc.gpsimd.memset(sel4[32 * c : 32 * c + 1, :], 1.0)
            gsb = cp.tile([P, S // P], f32)
            nc.sync.dma_start(gsb[:], gum.rearrange("(i p) -> p i", p=P))
            g16 = cp.tile([16, S // 16], f32)
            nc.sync.dma_start(g16[:], gum.rearrange("(f p) -> p f", p=16))
            io16i = cp.tile([16, S // 16], dt.int32)
            nc.gpsimd.iota(
                io16i[:], pattern=[[16, S // 16]], channel_multiplier=1, base=1
            )
            io16 = cp.tile([16, S // 16], f32)
            nc.vector.tensor_copy(io16[:], io16i[:])
            w1hi = cp.tile([P, ND, H], bf16)
            w1lo = cp.tile([P, ND, H], bf16)
            for jq in range(4):
                hsl = slice(jq * (H // 4), (jq + 1) * (H // 4))
                nc.sync.dma_start(w1hi[:, :, hsl], w1hi_d[:, :, hsl])
            for jq in range(4):
                hsl = slice(jq * (H // 4), (jq + 1) * (H // 4))
                nc.sync.dma_start(w1lo[:, :, hsl], w1lo_d[:, :, hsl])
            logits = cp.tile([1, S], f32)
            lscr = dramp.tile([S], f32)

            for _rep in range(n_rep):
              with (
                tc.tile_pool(name=f"xthl{_rep}", bufs=3) as xthlp,
                tc.tile_pool(name=f"ht{_rep}", bufs=1) as htp,
                tc.tile_pool(name=f"psg{_rep}", bufs=3, space="PSUM") as psg,
                tc.tile_pool(name=f"ps2{_rep}", bufs=2, space="PSUM") as ps2,
                tc.tile_pool(name=f"ps2b{_rep}", bufs=2, space="PSUM") as ps2b,
                tc.tile_pool(name=f"row{_rep}", bufs=2) as rowp,
              ):
                  for sc in range(NS):
                    xthi = xthlp.tile([P, ND, SC], bf16, tag="xthi")
                    xtlo = xthlp.tile([P, ND, SC], bf16, tag="xtlo")
                    nc.sync.dma_start(xthi[:], xthi_d[sc])
                    nc.sync.dma_start(xtlo[:], xtlo_d[sc])
                    ht = htp.tile([P, NH, SC], f32, tag="ht")
                    # GEMM2 partials: 4 col-groups of the PE array run M=1
                    # matmuls concurrently; rows {0,32,64,96} accumulate 4 j's
                    # each on top of a zeroed PSUM tile.
                    p2 = ps2.tile([P, SC], f32, tag="p2")
                    nc.vector.memset(p2[:], 0.0)
                    for j in range(NH):
                        pm = psg.tile([P, SC], f32, tag="pm")
                        one_term = "gemm1_1term" in ablate
                        for di in range(ND):
                            wsl = slice(j * P, (j + 1) * P)
                            nc.tensor.matmul(
                                pm[:], w1hi[:, di, wsl], xthi[:, di, :],
                                start=(di == 0), stop=(one_term and di == ND - 1),
                            )
                            if one_term:
                                continue
                            nc.tensor.matmul(
                                pm[:], w1hi[:, di, wsl], xtlo[:, di, :],
                                start=False, stop=False,
                            )
                            nc.tensor.matmul(
                                pm[:], w1lo[:, di, wsl], xthi[:, di, :],
                                start=False, stop=(di == ND - 1),
                            )
                        if "dve_relu" in ablate or j % 2 == 1:
                            nc.vector.tensor_scalar(
                                ht[:, j, :], pm[:], b1sb[:, j : j + 1], 0.0,
                                Alu.add, Alu.max,
                            )
                        else:
                            nc.scalar.activation(
                                ht[:, j, :], pm[:],
                                mybir.ActivationFunctionType.Relu,
                                bias=b1sb[:, j : j + 1],
                            )
                        if "gemm2" not in ablate:
                            c4 = 32 * (j % 4)
                            nc.tensor.matmul(
                                p2[c4 : c4 + 1, :], w2sb[:, j : j + 1], ht[:, j, :],
                                start=False, stop=False,
                                skip_group_check=True, tile_position=(0, c4),
                            )
                    if "gemm2" not in ablate:
                        rows = rowp.tile([P, SC], f32, tag="rows")
                        nc.vector.tensor_copy(rows[:], p2[:])
                        p2b = ps2b.tile([1, SC], f32, tag="p2b")
                        nc.tensor.matmul(
                            p2b[:], sel4[:, 0:1], rows[:], start=True, stop=True
                        )
                        nc.vector.tensor_copy(
                            logits[0:1, sc * SC : (sc + 1) * SC], p2b[:]
                        )
                    else:
                        nc.vector.memset(logits[0:1, sc * SC : (sc + 1) * SC], 0.1)

              # ---- top-k threshold + mask + sparse scatter ----
              # out is guaranteed pre-zeroed by the runner (native path zeros
              # ExternalOutputs; the PJRT path donates zero buffers), so only
              # the <=64 selected rows need to be written, via indirect DMA.
              with tc.tile_pool(name=f"topk{_rep}", bufs=1) as tkp:
                nc.sync.dma_start(lscr[None, :], logits[0:1, :])
                pert = tkp.tile([P, S // P], f32)
                nc.sync.dma_start(pert[:], lscr.rearrange("(i p) -> p i", p=P))
                nc.vector.tensor_tensor(pert[:], pert[:], gsb[:], Alu.add)
                ko = tkp.tile([P, 2], f32)
                q = 1.0 - (k_selected - 0.5) / (S - 1)
                nc.gpsimd.kth_largest(
                    ko[:], pert[:], n_per_lane=S // P, k=MAX_K + 8, quantile=q
                )
                thr = tkp.tile([P, 1], f32)
                nc.gpsimd.partition_broadcast(thr[:], ko[0:1, 1:2])
                maskt = tkp.tile([P, S // P], f32)
                nc.vector.tensor_scalar(
                    maskt[:], pert[:], thr[:, 0:1], None, Alu.is_gt
                )
                nc.sync.dma_start(
                    mask_out.rearrange("(i p) -> p i", p=P), maskt[:]
                )

                if "maskphase" not in ablate:
                    # mask in [16, S/16] layout (token t = f*16 + p), candidate
                    # ids cand = mask*(iota_base1) - 1, 128 OOB sentinels appended
                    F16 = S // 16
                    p16 = tkp.tile([16, F16], f32)
                    nc.sync.dma_start(p16[:], lscr.rearrange("(f p) -> p f", p=16))
                    nc.vector.tensor_tensor(p16[:], p16[:], g16[:], Alu.add)
                    m16 = tkp.tile([16, F16], f32)
                    nc.vector.tensor_scalar(
                        m16[:], p16[:], thr[:16, 0:1], None, Alu.is_gt
                    )
                    cand = tkp.tile([16, F16 + 8], f32)
                    nc.gpsimd.memset(cand[:], float(S))
                    nc.vector.tensor_tensor(
                        cand[:, :F16], m16[:], io16[:], Alu.mult
                    )
                    nc.vector.tensor_scalar_add(cand[:, :F16], cand[:, :F16], -1.0)
                    comp = tkp.tile([16, 8], f32)
                    nf = tkp.tile([1, 1], dt.uint32)
                    nc.gpsimd.sparse_gather(comp[:], cand[:], num_found=nf[:])
                    iscr = dramp.tile([P], f32)
                    nc.sync.dma_start(
                        iscr.rearrange("(f p) -> p f", p=16), comp[:]
                    )
                    idxf = tkp.tile([P, 1], f32)
                    nc.sync.dma_start(idxf[:], iscr[:, None])
                    idx = tkp.tile([P, 1], dt.int32)
                    nc.vector.tensor_copy(idx[:], idxf[:])
                    xg = tkp.tile([P, D], f32)
                    nc.gpsimd.indirect_dma_start(
                        out=xg[:],
                        out_offset=None,
                        in_=x[:],
                        in_offset=bass.IndirectOffsetOnAxis(ap=idx[:, :1], axis=0),
                        bounds_check=S - 1,
                        oob_is_err=False,
                    )
                    nc.gpsimd.indirect_dma_start(
                        out=out[:],
                        out_offset=bass.IndirectOffsetOnAxis(ap=idx[:, :1], axis=0),
                        in_=xg[:],
                        in_offset=None,
                        bounds_check=S - 1,
                        oob_is_err=False,
                    )

    nc.compile()
    return nc


def _split_transpose(xc):
    """[S, D] f32 -> (hi, lo) bf16 arrays laid out [NS, P, ND, SC]."""
    import ml_dtypes

    xt = np.ascontiguousarray(xc.T)  # [D, S]
    hi = xt.astype(ml_dtypes.bfloat16)
    lo = (xt - hi.astype(np.float32)).astype(ml_dtypes.bfloat16)

    def lay(a):
        # [D, S] = [(o p), (sc s)] -> [sc, p, o, s]
        return np.ascontiguousarray(
            a.reshape(ND, P, NS, SC).transpose(2, 1, 0, 3)
        )

    return lay(hi), lay(lo)


def _prep(token_embeddings, W1, b1, W2, b2, k_logits):
    k_gumbel, g2 = _host_prng()
    kl = np.asarray(k_logits, dtype=np.float32)
    k_pert = (kl + k_gumbel) / K_TAU
    e = np.exp((k_pert - k_pert.max()).astype(np.float32))
    k_soft = (e / e.sum()).astype(np.float32)
    expected_k = np.float32(
        np.sum(k_soft * np.arange(1, MAX_K + 1, dtype=np.float32))
    )
    k_selected = int(np.argmax(k_soft)) + 1

    import ml_dtypes

    te = np.ascontiguousarray(np.asarray(token_embeddings, dtype=np.float32))
    w1 = np.asarray(W1, dtype=np.float32).reshape(ND, P, H)  # [(o p), h] -> [o, p, h]
    w1hi = w1.astype(ml_dtypes.bfloat16)
    w1lo = (w1 - w1hi.astype(np.float32)).astype(ml_dtypes.bfloat16)
    w1hi = np.ascontiguousarray(w1hi.transpose(1, 0, 2))  # [p, o, h]
    w1lo = np.ascontiguousarray(w1lo.transpose(1, 0, 2))
    b1a = np.ascontiguousarray(np.asarray(b1, dtype=np.float32))
    w2 = np.ascontiguousarray(np.asarray(W2, dtype=np.float32).reshape(H))
    b2v = float(np.asarray(b2).reshape(-1)[0])
    gum = np.ascontiguousarray((g2 + b2v).astype(np.float32))

    in_maps = []
    for c in range(B):
        hi, lo = _split_transpose(te[c])
        in_maps.append(
            {
                "x": te[c],
                "xthi": hi,
                "xtlo": lo,
                "w1hi": w1hi,
                "w1lo": w1lo,
                "b1": b1a,
                "w2": w2,
                "gum": gum[c],
            }
        )
    return in_maps, expected_k, k_selected


def kernel(token_embeddings, W1, b1, W2, b2, k_logits):
    from concourse import bass_utils

    in_maps, expected_k, k_selected = _prep(
        token_embeddings, W1, b1, W2, b2, k_logits
    )
    if k_selected not in _cache:
        _cache[k_selected] = _build(k_selected)
    nc = _cache[k_selected]

    res = bass_utils.run_bass_kernel_spmd(nc, in_maps, core_ids=list(range(B)))
    filtered = np.stack([res.results[c]["out"] for c in range(B)])
    selection_mask = np.stack([res.results[c]["mask"] for c in range(B)])
    return filtered, selection_mask, expected_k


# revision 43
# speedup vs baseline: 1.1756x; 1.1756x over previous
"""AdaptiveTokenFilter Trainium2 kernel.

kernel(**inputs) takes the FULL inputs (token_embeddings [8,4096,1024], W1
[1024,2048], b1 [2048], W2 [2048,1], b2 [1], k_logits [64]) and returns
(filtered_embeddings, selection_mask, expected_k) matching the reference.

Strategy: data-parallel over batch — one NeuronCore per batch row.
Per core: the scorer GEMM1 ([4096,1024]@[1024,2048]) runs as a 3-term
bf16 hi/lo-split matmul (fp32-level fidelity at bf16 TensorE speed; the
hi/lo split and the [D,S] transpose are host-side data prep), relu+bias
fused into the PSUM->SBUF copy, GEMM2 ([.,2048]@[2048,1]) in fp32
interleaved into the GEMM1 tile loop, gumbel perturbation added on-chip
(host-generated noise, bit-identical to the reference's jax threefry
stream), exact top-k threshold via the gpsimd kth_largest instruction,
and the filtered output produced sparsely: outputs are pre-zeroed by the
runner, selected token ids are compacted on-chip (iota/mask/sparse_gather
with OOB sentinels), and only the <=64 selected rows of X are
gather/scattered into the output via indirect DMA.

The learnable-k branch (k_selected, expected_k) is a 64-element
computation done on host; k_selected is baked into the device program.
"""

import numpy as np

S, D, H, P = 4096, 1024, 2048, 128
B = 8
NS, ND, NH, SC = 8, 8, 16, 512  # S-chunks, D-tiles, H-tiles, chunk size
MAX_K = 64
TAU = 1.0
K_TAU = 1.0

_cache = {}


def _host_prng():
    """Reproduce the reference's jax PRNG stream bit-exactly on CPU."""
    import jax
    import jax.numpy as jnp

    cpu = jax.devices("cpu")[0]
    with jax.default_device(cpu):
        rng = jax.random.key(42)
        rng1, rng2 = jax.random.split(rng)

        def _gumbel(r, shape):
            u = jax.random.uniform(r, shape, minval=1e-08, maxval=1.0 - 1e-08)
            return -jnp.log(-jnp.log(u))

        k_gumbel = np.asarray(_gumbel(rng1, (MAX_K,)))
        g2 = np.asarray(_gumbel(rng2, (B, S)))
    return k_gumbel, g2


def _build(k_selected: int, n_rep: int = 1, ablate: frozenset = frozenset()):
    import concourse.bass as bass  # noqa: F401
    import concourse.mybir as mybir
    import concourse.tile as tile
    from concourse import bacc

    dt = mybir.dt
    f32, bf16 = dt.float32, dt.bfloat16
    Alu = mybir.AluOpType

    nc = bacc.Bacc("TRN2", target_bir_lowering=False, debug=False)
    x = nc.dram_tensor("x", [S, D], f32, kind="ExternalInput").ap()
    # pre-transposed bf16 hi/lo split of x, chunk-major: [sc, p, d_outer, s_in]
    xthi_d = nc.dram_tensor("xthi", [NS, P, ND, SC], bf16, kind="ExternalInput").ap()
    xtlo_d = nc.dram_tensor("xtlo", [NS, P, ND, SC], bf16, kind="ExternalInput").ap()
    w1hi_d = nc.dram_tensor("w1hi", [P, ND, H], bf16, kind="ExternalInput").ap()
    w1lo_d = nc.dram_tensor("w1lo", [P, ND, H], bf16, kind="ExternalInput").ap()
    b1 = nc.dram_tensor("b1", [H], f32, kind="ExternalInput").ap()
    w2 = nc.dram_tensor("w2", [H], f32, kind="ExternalInput").ap()
    gum = nc.dram_tensor("gum", [S], f32, kind="ExternalInput").ap()
    out = nc.dram_tensor("out", [S, D], f32, kind="ExternalOutput").ap()
    mask_out = nc.dram_tensor("mask", [S], f32, kind="ExternalOutput").ap()

    with tile.TileContext(nc) as tc:
        with (
            tc.tile_pool(name="const", bufs=1) as cp,
            tc.tile_pool(name="dram", bufs=1, space="DRAM") as dramp,
        ):
            b1sb = cp.tile([P, NH], f32)
            nc.sync.dma_start(b1sb[:], b1.rearrange("(o p) -> p o", p=P))
            w2sb = cp.tile([P, NH], f32)
            nc.sync.dma_start(w2sb[:], w2.rearrange("(o p) -> p o", p=P))
            sel4 = cp.tile([P, 1], f32)
            nc.gpsimd.memset(sel4[:], 0.0)
            for c in range(4):
                nc.gpsimd.memset(sel4[32 * c : 32 * c + 1, :], 1.0)
            gsb = cp.tile([P, S // P], f32)
            nc.sync.dma_start(gsb[:], gum.rearrange("(i p) -> p i", p=P))
            g16 = cp.tile([16, S // 16], f32)
            nc.sync.dma_start(g16[:], gum.rearrange("(f p) -> p f", p=16))
            io16i = cp.tile([16, S // 16], dt.int32)
            nc.gpsimd.iota(
                io16i[:], pattern=[[16, S // 16]], channel_multiplier=1, base=1
            )
            io16 = cp.tile([16, S // 16], f32)
            nc.vector.tensor_copy(io16[:], io16i[:])
            w1hi = cp.tile([P, ND, H], bf16)
            w1lo = cp.tile([P, ND, H], bf16)
            for jq in range(4):
                hsl = slice(jq * (H // 4), (jq + 1) * (H // 4))
                nc.sync.dma_start(w1hi[:, :, hsl], w1hi_d[:, :, hsl])
            for jq in range(4):
                hsl = slice(jq * (H // 4), (jq + 1) * (H // 4))
                nc.sync.dma_start(w1lo[:, :, hsl], w1lo_d[:, :, hsl])
            pert_pre = cp.tile([P, S // P], f32)
            p16_pre = cp.tile([16, S // 16], f32)
            lscr = dramp.tile([S], f32)

            for _rep in range(n_rep):
              with (
                tc.tile_pool(name=f"xthl{_rep}", bufs=3) as xthlp,
                tc.tile_pool(name=f"ht{_rep}", bufs=1) as htp,
                tc.tile_pool(name=f"psg{_rep}", bufs=3, space="PSUM") as psg,
                tc.tile_pool(name=f"ps2{_rep}", bufs=2, space="PSUM") as ps2,
                tc.tile_pool(name=f"ps2b{_rep}", bufs=2, space="PSUM") as ps2b,
                tc.tile_pool(name=f"row{_rep}", bufs=2) as rowp,
              ):
                  for sc in range(NS):
                    xthi = xthlp.tile([P, ND, SC], bf16, tag="xthi")
                    xtlo = xthlp.tile([P, ND, SC], bf16, tag="xtlo")
                    nc.sync.dma_start(xthi[:], xthi_d[sc])
                    nc.sync.dma_start(xtlo[:], xtlo_d[sc])
                    ht = htp.tile([P, NH, SC], f32, tag="ht")
                    # GEMM2 partials: 4 col-groups of the PE array run M=1
                    # matmuls concurrently; rows {0,32,64,96} accumulate 4 j's
                    # each on top of a zeroed PSUM tile.
                    p2 = ps2.tile([P, SC], f32, tag="p2")
                    nc.vector.memset(p2[:], 0.0)
                    for j in range(NH):
                        pm = psg.tile([P, SC], f32, tag="pm")
                        one_term = "gemm1_1term" in ablate
                        for di in range(ND):
                            wsl = slice(j * P, (j + 1) * P)
                            nc.tensor.matmul(
                                pm[:], w1hi[:, di, wsl], xthi[:, di, :],
                                start=(di == 0), stop=(one_term and di == ND - 1),
                            )
                            if one_term:
                                continue
                            nc.tensor.matmul(
                                pm[:], w1hi[:, di, wsl], xtlo[:, di, :],
                                start=False, stop=False,
                            )
                            nc.tensor.matmul(
                                pm[:], w1lo[:, di, wsl], xthi[:, di, :],
                                start=False, stop=(di == ND - 1),
                            )
                        if "dve_relu" in ablate or j % 2 == 1:
                            nc.vector.tensor_scalar(
                                ht[:, j, :], pm[:], b1sb[:, j : j + 1], 0.0,
                                Alu.add, Alu.max,
                            )
                        else:
                            nc.scalar.activation(
                                ht[:, j, :], pm[:],
                                mybir.ActivationFunctionType.Relu,
                                bias=b1sb[:, j : j + 1],
                            )
                        if "gemm2" not in ablate:
                            if "gemm2_serial" in ablate:
                                nc.tensor.matmul(
                                    p2[0:1, :], w2sb[:, j : j + 1], ht[:, j, :],
                                    start=False, stop=False, skip_group_check=True,
                                )
                            else:
                                c4 = 32 * (j % 4)
                                nc.tensor.matmul(
                                    p2[c4 : c4 + 1, :], w2sb[:, j : j + 1],
                                    ht[:, j, :], start=False, stop=False,
                                    skip_group_check=True, tile_position=(0, c4),
                                )
                    rows = rowp.tile([P, SC], f32, tag="rows")
                    nc.vector.tensor_copy(rows[:], p2[:])
                    p2b = ps2b.tile([1, SC], f32, tag="p2b")
                    nc.tensor.matmul(
                        p2b[:], sel4[:, 0:1], rows[:], start=True, stop=True
                    )
                    lgc = rowp.tile([1, SC], f32, tag="lgc")
                    nc.vector.tensor_copy(lgc[:], p2b[:])
                    # ship this chunk's logits out and reload into both top-k
                    # layouts while later chunks compute
                    lslice = lscr[sc * SC : (sc + 1) * SC]
                    nc.sync.dma_start(lslice[None, :], lgc[:])
                    nc.sync.dma_start(
                        pert_pre[:, sc * (SC // P) : (sc + 1) * (SC // P)],
                        lslice.rearrange("(i p) -> p i", p=P),
                    )
                    nc.sync.dma_start(
                        p16_pre[:, sc * (SC // 16) : (sc + 1) * (SC // 16)],
                        lslice.rearrange("(f p) -> p f", p=16),
                    )

              # ---- top-k threshold + mask + sparse scatter ----
              # out is guaranteed pre-zeroed by the runner (native path zeros
              # ExternalOutputs; the PJRT path donates zero buffers), so only
              # the <=64 selected rows need to be written, via indirect DMA.
              with tc.tile_pool(name=f"topk{_rep}", bufs=1) as tkp:
                pert = tkp.tile([P, S // P], f32)
                nc.vector.tensor_tensor(pert[:], pert_pre[:], gsb[:], Alu.add)
                ko = tkp.tile([P, 2], f32)
                q = 1.0 - (k_selected - 0.5) / (S - 1)
                nc.gpsimd.kth_largest(
                    ko[:], pert[:], n_per_lane=S // P, k=MAX_K + 8, quantile=q
                )
                thr = tkp.tile([P, 1], f32)
                nc.gpsimd.partition_broadcast(thr[:], ko[0:1, 1:2])
                maskt = tkp.tile([P, S // P], f32)
                nc.vector.tensor_scalar(
                    maskt[:], pert[:], thr[:, 0:1], None, Alu.is_gt
                )
                nc.sync.dma_start(
                    mask_out.rearrange("(i p) -> p i", p=P), maskt[:]
                )

                if "maskphase" not in ablate:
                    # mask in [16, S/16] layout (token t = f*16 + p), candidate
                    # ids cand = mask*(iota_base1) - 1, 128 OOB sentinels appended
                    F16 = S // 16
                    p16 = tkp.tile([16, F16], f32)
                    nc.vector.tensor_tensor(p16[:], p16_pre[:], g16[:], Alu.add)
                    m16 = tkp.tile([16, F16], f32)
                    nc.vector.tensor_scalar(
                        m16[:], p16[:], thr[:16, 0:1], None, Alu.is_gt
                    )
                    cand = tkp.tile([16, F16 + 8], f32)
                    nc.gpsimd.memset(cand[:], float(S))
                    nc.vector.tensor_tensor(
                        cand[:, :F16], m16[:], io16[:], Alu.mult
                    )
                    nc.vector.tensor_scalar_add(cand[:, :F16], cand[:, :F16], -1.0)
                    comp = tkp.tile([16, 8], f32)
                    nf = tkp.tile([1, 1], dt.uint32)
                    nc.gpsimd.sparse_gather(comp[:], cand[:], num_found=nf[:])
                    iscr = dramp.tile([P], f32)
                    nc.sync.dma_start(
                        iscr.rearrange("(f p) -> p f", p=16), comp[:]
                    )
                    idxf = tkp.tile([P, 1], f32)
                    nc.sync.dma_start(idxf[:], iscr[:, None])
                    idx = tkp.tile([P, 1], dt.int32)
                    nc.vector.tensor_copy(idx[:], idxf[:])
                    xg = tkp.tile([P, D], f32)
                    nc.gpsimd.indirect_dma_start(
                        out=xg[:],
                        out_offset=None,
                        in_=x[:],
                        in_offset=bass.IndirectOffsetOnAxis(ap=idx[:, :1], axis=0),
                        bounds_check=S - 1,
                        oob_is_err=False,
                    )
                    nc.gpsimd.indirect_dma_start(
                        out=out[:],
                        out_offset=bass.IndirectOffsetOnAxis(ap=idx[:, :1], axis=0),
                        in_=xg[:],
                        in_offset=None,
                        bounds_check=S - 1,
                        oob_is_err=False,
                    )

    nc.compile()
    return nc


def _split_transpose(xc):
    """[S, D] f32 -> (hi, lo) bf16 arrays laid out [NS, P, ND, SC]."""
    import ml_dtypes

    xt = np.ascontiguousarray(xc.T)  # [D, S]
    hi = xt.astype(ml_dtypes.bfloat16)
    lo = (xt - hi.astype(np.float32)).astype(ml_dtypes.bfloat16)

    def lay(a):
        # [D, S] = [(o p), (sc s)] -> [sc, p, o, s]
        return np.ascontiguousarray(
            a.reshape(ND, P, NS, SC).transpose(2, 1, 0, 3)
        )

    return lay(hi), lay(lo)


def _prep(token_embeddings, W1, b1, W2, b2, k_logits):
    k_gumbel, g2 = _host_prng()
    kl = np.asarray(k_logits, dtype=np.float32)
    k_pert = (kl + k_gumbel) / K_TAU
    e = np.exp((k_pert - k_pert.max()).astype(np.float32))
    k_soft = (e / e.sum()).astype(np.float32)
    expected_k = np.float32(
        np.sum(k_soft * np.arange(1, MAX_K + 1, dtype=np.float32))
    )
    k_selected = int(np.argmax(k_soft)) + 1

    import ml_dtypes

    te = np.ascontiguousarray(np.asarray(token_embeddings, dtype=np.float32))
    w1 = np.asarray(W1, dtype=np.float32).reshape(ND, P, H)  # [(o p), h] -> [o, p, h]
    w1hi = w1.astype(ml_dtypes.bfloat16)
    w1lo = (w1 - w1hi.astype(np.float32)).astype(ml_dtypes.bfloat16)
    w1hi = np.ascontiguousarray(w1hi.transpose(1, 0, 2))  # [p, o, h]
    w1lo = np.ascontiguousarray(w1lo.transpose(1, 0, 2))
    b1a = np.ascontiguousarray(np.asarray(b1, dtype=np.float32))
    w2 = np.ascontiguousarray(np.asarray(W2, dtype=np.float32).reshape(H))
    b2v = float(np.asarray(b2).reshape(-1)[0])
    gum = np.ascontiguousarray((g2 + b2v).astype(np.float32))

    in_maps = []
    for c in range(B):
        hi, lo = _split_transpose(te[c])
        in_maps.append(
            {
                "x": te[c],
                "xthi": hi,
                "xtlo": lo,
                "w1hi": w1hi,
                "w1lo": w1lo,
                "b1": b1a,
                "w2": w2,
                "gum": gum[c],
            }
        )
    return in_maps, expected_k, k_selected


def kernel(token_embeddings, W1, b1, W2, b2, k_logits):
    from concourse import bass_utils

    in_maps, expected_k, k_selected = _prep(
        token_embeddings, W1, b1, W2, b2, k_logits
    )
    if k_selected not in _cache:
        _cache[k_selected] = _build(k_selected)
    nc = _cache[k_selected]

    res = bass_utils.run_bass_kernel_spmd(nc, in_maps, core_ids=list(range(B)))
    filtered = np.stack([res.results[c]["out"] for c in range(B)])
    selection_mask = np.stack([res.results[c]["mask"] for c in range(B)])
    return filtered, selection_mask, expected_k


# revision 45
# speedup vs baseline: 1.4798x; 1.2587x over previous
"""AdaptiveTokenFilter Trainium2 kernel.

kernel(**inputs) takes the FULL inputs (token_embeddings [8,4096,1024], W1
[1024,2048], b1 [2048], W2 [2048,1], b2 [1], k_logits [64]) and returns
(filtered_embeddings, selection_mask, expected_k) matching the reference.

Strategy: data-parallel over batch — one NeuronCore per batch row.
Per core: the scorer GEMM1 ([4096,1024]@[1024,2048]) runs as a 3-term
bf16 hi/lo-split matmul (fp32-level fidelity at bf16 TensorE speed; the
hi/lo split and the [D,S] transpose are host-side data prep), relu+bias
fused into the PSUM->SBUF copy, GEMM2 ([.,2048]@[2048,1]) in fp32
interleaved into the GEMM1 tile loop, gumbel perturbation added on-chip
(host-generated noise, bit-identical to the reference's jax threefry
stream), exact top-k threshold via the gpsimd kth_largest instruction,
and the filtered output produced sparsely: outputs are pre-zeroed by the
runner, selected token ids are compacted on-chip (iota/mask/sparse_gather
with OOB sentinels), and only the <=64 selected rows of X are
gather/scattered into the output via indirect DMA.

The learnable-k branch (k_selected, expected_k) is a 64-element
computation done on host; k_selected is baked into the device program.
"""

import numpy as np

S, D, H, P = 4096, 1024, 2048, 128
B = 8
NS, ND, NH, SC = 8, 8, 16, 512  # S-chunks, D-tiles, H-tiles, chunk size
MAX_K = 64
TAU = 1.0
K_TAU = 1.0

_cache = {}


def _host_prng():
    """Reproduce the reference's jax PRNG stream bit-exactly on CPU."""
    import jax
    import jax.numpy as jnp

    cpu = jax.devices("cpu")[0]
    with jax.default_device(cpu):
        rng = jax.random.key(42)
        rng1, rng2 = jax.random.split(rng)

        def _gumbel(r, shape):
            u = jax.random.uniform(r, shape, minval=1e-08, maxval=1.0 - 1e-08)
            return -jnp.log(-jnp.log(u))

        k_gumbel = np.asarray(_gumbel(rng1, (MAX_K,)))
        g2 = np.asarray(_gumbel(rng2, (B, S)))
    return k_gumbel, g2


def _build(k_selected: int, n_rep: int = 1, ablate: frozenset = frozenset()):
    import concourse.bass as bass  # noqa: F401
    import concourse.mybir as mybir
    import concourse.tile as tile
    from concourse import bacc

    dt = mybir.dt
    f32, bf16 = dt.float32, dt.bfloat16
    Alu = mybir.AluOpType

    nc = bacc.Bacc("TRN2", target_bir_lowering=False, debug=False)
    x = nc.dram_tensor("x", [S, D], f32, kind="ExternalInput").ap()
    # pre-transposed bf16 hi/lo split of x, chunk-major: [sc, p, d_outer, s_in]
    xthi_d = nc.dram_tensor("xthi", [NS, P, ND, SC], bf16, kind="ExternalInput").ap()
    xtlo_d = nc.dram_tensor("xtlo", [NS, P, ND, SC], bf16, kind="ExternalInput").ap()
    w1hi_d = nc.dram_tensor("w1hi", [P, ND, H], bf16, kind="ExternalInput").ap()
    w1lo_d = nc.dram_tensor("w1lo", [P, ND, H], bf16, kind="ExternalInput").ap()
    b1 = nc.dram_tensor("b1", [H], f32, kind="ExternalInput").ap()
    w2 = nc.dram_tensor("w2", [H], f32, kind="ExternalInput").ap()
    gum = nc.dram_tensor("gum", [S], f32, kind="ExternalInput").ap()
    out = nc.dram_tensor("out", [S, D], f32, kind="ExternalOutput").ap()
    mask_out = nc.dram_tensor("mask", [S], f32, kind="ExternalOutput").ap()

    with tile.TileContext(nc) as tc:
        with (
            tc.tile_pool(name="const", bufs=1) as cp,
            tc.tile_pool(name="dram", bufs=1, space="DRAM") as dramp,
        ):
            b1sb = cp.tile([P, NH], f32)
            nc.sync.dma_start(b1sb[:], b1.rearrange("(o p) -> p o", p=P))
            w2sb = cp.tile([P, NH], f32)
            nc.sync.dma_start(w2sb[:], w2.rearrange("(o p) -> p o", p=P))
            sel4 = cp.tile([P, 1], f32)
            nc.gpsimd.memset(sel4[:], 0.0)
            for c in range(4):
                nc.gpsimd.memset(sel4[32 * c : 32 * c + 1, :], 1.0)
            gsb = cp.tile([P, S // P], f32)
            nc.sync.dma_start(gsb[:], gum.rearrange("(i p) -> p i", p=P))
            g16 = cp.tile([16, S // 16], f32)
            nc.sync.dma_start(g16[:], gum.rearrange("(f p) -> p f", p=16))
            io16i = cp.tile([16, S // 16], dt.int32)
            nc.gpsimd.iota(
                io16i[:], pattern=[[16, S // 16]], channel_multiplier=1, base=1
            )
            io16 = cp.tile([16, S // 16], f32)
            nc.vector.tensor_copy(io16[:], io16i[:])
            w1hi = cp.tile([P, ND, H], bf16)
            w1lo = cp.tile([P, ND, H], bf16)
            for jq in range(4):
                hsl = slice(jq * (H // 4), (jq + 1) * (H // 4))
                nc.sync.dma_start(w1hi[:, :, hsl], w1hi_d[:, :, hsl])
            for jq in range(4):
                hsl = slice(jq * (H // 4), (jq + 1) * (H // 4))
                nc.sync.dma_start(w1lo[:, :, hsl], w1lo_d[:, :, hsl])
            pert_pre = cp.tile([P, S // P], f32)
            p16_pre = cp.tile([16, S // 16], f32)
            lscr = dramp.tile([S], f32)

            for _rep in range(n_rep):
              with (
                tc.tile_pool(
                    name=f"xthl{_rep}", bufs=(4 if "xthl4" in ablate else 3)
                ) as xthlp,
                tc.tile_pool(name=f"ht{_rep}", bufs=1) as htp,
                tc.tile_pool(
                    name=f"psg{_rep}",
                    bufs=(4 if "psg4" in ablate else 3),
                    space="PSUM",
                ) as psg,
                tc.tile_pool(name=f"ps2{_rep}", bufs=2, space="PSUM") as ps2,
                tc.tile_pool(name=f"ps2b{_rep}", bufs=2, space="PSUM") as ps2b,
                tc.tile_pool(name=f"row{_rep}", bufs=2) as rowp,
              ):
                  for sc in range(NS):
                    xthi = xthlp.tile([P, ND, SC], bf16, tag="xthi")
                    xtlo = xthlp.tile([P, ND, SC], bf16, tag="xtlo")
                    nc.sync.dma_start(xthi[:], xthi_d[sc])
                    nc.sync.dma_start(xtlo[:], xtlo_d[sc])
                    ht = htp.tile([P, NH, SC], f32, tag="ht")
                    # GEMM2 partials: 4 col-groups of the PE array run M=1
                    # matmuls concurrently; rows {0,32,64,96} accumulate 4 j's
                    # each on top of a zeroed PSUM tile.
                    p2 = ps2.tile([P, SC], f32, tag="p2")
                    nc.vector.memset(p2[:], 0.0)
                    for j in range(NH):
                        pm = psg.tile([P, SC], f32, tag="pm")
                        one_term = "gemm1_1term" in ablate
                        for di in range(ND):
                            wsl = slice(j * P, (j + 1) * P)
                            nc.tensor.matmul(
                                pm[:], w1hi[:, di, wsl], xthi[:, di, :],
                                start=(di == 0), stop=(one_term and di == ND - 1),
                            )
                            if one_term:
                                continue
                            nc.tensor.matmul(
                                pm[:], w1hi[:, di, wsl], xtlo[:, di, :],
                                start=False, stop=False,
                            )
                            nc.tensor.matmul(
                                pm[:], w1lo[:, di, wsl], xthi[:, di, :],
                                start=False, stop=(di == ND - 1),
                            )
                        if "dve_relu" in ablate or j % 2 == 1:
                            nc.vector.tensor_scalar(
                                ht[:, j, :], pm[:], b1sb[:, j : j + 1], 0.0,
                                Alu.add, Alu.max,
                            )
                        else:
                            nc.scalar.activation(
                                ht[:, j, :], pm[:],
                                mybir.ActivationFunctionType.Relu,
                                bias=b1sb[:, j : j + 1],
                            )
                        if "gemm2" not in ablate:
                            if "gemm2_serial" in ablate:
                                nc.tensor.matmul(
                                    p2[0:1, :], w2sb[:, j : j + 1], ht[:, j, :],
                                    start=False, stop=False, skip_group_check=True,
                                )
                            else:
                                c4 = 32 * (j % 4)
                                nc.tensor.matmul(
                                    p2[c4 : c4 + 1, :], w2sb[:, j : j + 1],
                                    ht[:, j, :], start=False, stop=False,
                                    skip_group_check=True, tile_position=(0, c4),
                                )
                    rows = rowp.tile([P, SC], f32, tag="rows")
                    nc.vector.tensor_copy(rows[:], p2[:])
                    p2b = ps2b.tile([1, SC], f32, tag="p2b")
                    nc.tensor.matmul(
                        p2b[:], sel4[:, 0:1], rows[:], start=True, stop=True
                    )
                    lgc = rowp.tile([1, SC], f32, tag="lgc")
                    nc.vector.tensor_copy(lgc[:], p2b[:])
                    # ship this chunk's logits out and reload into both top-k
                    # layouts while later chunks compute
                    lslice = lscr[sc * SC : (sc + 1) * SC]
                    nc.sync.dma_start(lslice[None, :], lgc[:])
                    nc.sync.dma_start(
                        pert_pre[:, sc * (SC // P) : (sc + 1) * (SC // P)],
                        lslice.rearrange("(i p) -> p i", p=P),
                    )
                    nc.sync.dma_start(
                        p16_pre[:, sc * (SC // 16) : (sc + 1) * (SC // 16)],
                        lslice.rearrange("(f p) -> p f", p=16),
                    )

              # ---- top-k threshold + mask + sparse scatter ----
              # out is guaranteed pre-zeroed by the runner (native path zeros
              # ExternalOutputs; the PJRT path donates zero buffers), so only
              # the <=64 selected rows need to be written, via indirect DMA.
              with tc.tile_pool(name=f"topk{_rep}", bufs=1) as tkp:
                pert = tkp.tile([P, S // P], f32)
                nc.vector.tensor_tensor(pert[:], pert_pre[:], gsb[:], Alu.add)
                ko = tkp.tile([P, 2], f32)
                q = 1.0 - (k_selected - 0.5) / (S - 1)
                nc.gpsimd.kth_largest(
                    ko[:], pert[:], n_per_lane=S // P, k=MAX_K + 8, quantile=q
                )
                thr = tkp.tile([P, 1], f32)
                nc.gpsimd.partition_broadcast(thr[:], ko[0:1, 1:2])
                maskt = tkp.tile([P, S // P], f32)
                nc.vector.tensor_scalar(
                    maskt[:], pert[:], thr[:, 0:1], None, Alu.is_gt
                )
                nc.sync.dma_start(
                    mask_out.rearrange("(i p) -> p i", p=P), maskt[:]
                )

                if "maskphase" not in ablate:
                    # mask in [16, S/16] layout (token t = f*16 + p), candidate
                    # ids cand = mask*(iota_base1) - 1, 128 OOB sentinels appended
                    F16 = S // 16
                    p16 = tkp.tile([16, F16], f32)
                    nc.vector.tensor_tensor(p16[:], p16_pre[:], g16[:], Alu.add)
                    m16 = tkp.tile([16, F16], f32)
                    nc.vector.tensor_scalar(
                        m16[:], p16[:], thr[:16, 0:1], None, Alu.is_gt
                    )
                    cand = tkp.tile([16, F16 + 8], f32)
                    nc.gpsimd.memset(cand[:], float(S))
                    nc.vector.tensor_tensor(
                        cand[:, :F16], m16[:], io16[:], Alu.mult
                    )
                    nc.vector.tensor_scalar_add(cand[:, :F16], cand[:, :F16], -1.0)
                    comp = tkp.tile([16, 8], f32)
                    nf = tkp.tile([1, 1], dt.uint32)
                    nc.gpsimd.sparse_gather(comp[:], cand[:], num_found=nf[:])
                    iscr = dramp.tile([P], f32)
                    nc.sync.dma_start(
                        iscr.rearrange("(f p) -> p f", p=16), comp[:]
                    )
                    idxf = tkp.tile([P, 1], f32)
                    nc.sync.dma_start(idxf[:], iscr[:, None])
                    idx = tkp.tile([P, 1], dt.int32)
                    nc.vector.tensor_copy(idx[:], idxf[:])
                    xg = tkp.tile([P, D], f32)
                    nc.gpsimd.indirect_dma_start(
                        out=xg[:],
                        out_offset=None,
                        in_=x[:],
                        in_offset=bass.IndirectOffsetOnAxis(ap=idx[:, :1], axis=0),
                        bounds_check=S - 1,
                        oob_is_err=False,
                    )
                    nc.gpsimd.indirect_dma_start(
                        out=out[:],
                        out_offset=bass.IndirectOffsetOnAxis(ap=idx[:, :1], axis=0),
                        in_=xg[:],
                        in_offset=None,
                        bounds_check=S - 1,
                        oob_is_err=False,
                    )

    nc.compile()
    return nc


def _split_transpose(xc):
    """[S, D] f32 -> (hi, lo) bf16 arrays laid out [NS, P, ND, SC]."""
    import ml_dtypes

    xt = np.ascontiguousarray(xc.T)  # [D, S]
    hi = xt.astype(ml_dtypes.bfloat16)
    lo = (xt - hi.astype(np.float32)).astype(ml_dtypes.bfloat16)

    def lay(a):
        # [D, S] = [(o p), (sc s)] -> [sc, p, o, s]
        return np.ascontiguousarray(
            a.reshape(ND, P, NS, SC).transpose(2, 1, 0, 3)
        )

    return lay(hi), lay(lo)


def _prep(token_embeddings, W1, b1, W2, b2, k_logits):
    k_gumbel, g2 = _host_prng()
    kl = np.asarray(k_logits, dtype=np.float32)
    k_pert = (kl + k_gumbel) / K_TAU
    e = np.exp((k_pert - k_pert.max()).astype(np.float32))
    k_soft = (e / e.sum()).astype(np.float32)
    expected_k = np.float32(
        np.sum(k_soft * np.arange(1, MAX_K + 1, dtype=np.float32))
    )
    k_selected = int(np.argmax(k_soft)) + 1

    import ml_dtypes

    te = np.ascontiguousarray(np.asarray(token_embeddings, dtype=np.float32))
    w1 = np.asarray(W1, dtype=np.float32).reshape(ND, P, H)  # [(o p), h] -> [o, p, h]
    w1hi = w1.astype(ml_dtypes.bfloat16)
    w1lo = (w1 - w1hi.astype(np.float32)).astype(ml_dtypes.bfloat16)
    w1hi = np.ascontiguousarray(w1hi.transpose(1, 0, 2))  # [p, o, h]
    w1lo = np.ascontiguousarray(w1lo.transpose(1, 0, 2))
    b1a = np.ascontiguousarray(np.asarray(b1, dtype=np.float32))
    w2 = np.ascontiguousarray(np.asarray(W2, dtype=np.float32).reshape(H))
    b2v = float(np.asarray(b2).reshape(-1)[0])
    gum = np.ascontiguousarray((g2 + b2v).astype(np.float32))

    in_maps = []
    for c in range(B):
        hi, lo = _split_transpose(te[c])
        in_maps.append(
            {
                "x": te[c],
                "xthi": hi,
                "xtlo": lo,
                "w1hi": w1hi,
                "w1lo": w1lo,
                "b1": b1a,
                "w2": w2,
                "gum": gum[c],
            }
        )
    return in_maps, expected_k, k_selected


def kernel(token_embeddings, W1, b1, W2, b2, k_logits):
    from concourse import bass_utils

    in_maps, expected_k, k_selected = _prep(
        token_embeddings, W1, b1, W2, b2, k_logits
    )
    if k_selected not in _cache:
        _cache[k_selected] = _build(k_selected)
    nc = _cache[k_selected]

    res = bass_utils.run_bass_kernel_spmd(nc, in_maps, core_ids=list(range(B)))
    filtered = np.stack([res.results[c]["out"] for c in range(B)])
    selection_mask = np.stack([res.results[c]["mask"] for c in range(B)])
    return filtered, selection_mask, expected_k
